# revision 49
# baseline (speedup 1.0000x reference)
"""Multi-head GQA attention (B=2, S=2048, D=2048, H=16, KVH=4) on 8 TRN2
NeuronCores.

Sharding: core i = (b, g) with b = i // 4 (batch), g = i % 4 (sequence
chunk of 512 queries). Each core computes Q for its 512 queries over all
16 heads, K/V for its own 512 sequence positions, AllGathers K/V within
its 4-core batch group, then runs full attention + output projection for
its query chunk. Host concatenates the 8 [512, 2048] chunks.

Layout strategy (no on-chip transposes):
 - host passes x transposed per chunk (xT [D, 512]) so projections
   computed as w.T @ xT yield QT/KT with head-dim on partitions —
   exactly the operand layout attention needs.
 - wq/wk columns permuted per head (even dims first, odd second) so RoPE
   halves are contiguous partition ranges [0:64)/[64:128). Scores are
   permutation-invariant since q and k are permuted identically.
 - scores computed transposed (ST[k, q] = KT.T @ QT), exp'd on ScalarE
   straight out of PSUM two k-tiles at a time (scale=1/sqrt(HD) folded
   in, no max-subtraction: scores are O(10) so f32 exp is safe), giving
   probs in the [k, q] layout the AV matmul wants as its moving operand.
 - softmax denominator: DVE pairwise add-tree over the 16 prob tiles
   (bf16, 2x mode), then a single all-ones [k,128] stationary matmul on
   the tree root replicates the denominator across partitions, so
   normalization is reciprocal + elementwise multiply, no broadcast.
   (One matmul per head instead of 16 — the PE is the bottleneck.)
 - weights are host-pretiled so every DMA is a contiguous block.
 - a short stream of dummy matmuls on zeroed SBUF warms the PE clock
   gate (HAM) during the initial input DMAs, so real matmuls start at
   full clock.
"""

import numpy as np
import ml_dtypes

B, S, D = 2, 2048, 2048
H, KVH = 16, 4
HD = D // H            # 128
R = H // KVH           # 4 (GQA repeat)
NCORES = 8
G = 4                  # cores per batch group = seq chunks
SQ = S // G            # 512 queries/keys per core chunk
DKV = KVH * HD         # 512
KS = D // 128          # 16 contraction slices
NKT = S // 128         # 16 key tiles
NPR = NKT // 2         # 8 key-tile pairs
SCALE = 1.0 / float(np.sqrt(HD))

_CACHE = {}


def _build_nc():
    import concourse.tile as tile
    from concourse import bacc, mybir
    from contextlib import ExitStack

    f32 = mybir.dt.float32
    bf = mybir.dt.bfloat16
    AF = mybir.ActivationFunctionType

    nc = bacc.Bacc("TRN2", target_bir_lowering=False, debug=False, num_devices=NCORES)

    xt_d = nc.dram_tensor("xt", [128, KS * SQ], bf, kind="ExternalInput")
    wq_d = nc.dram_tensor("wq", [H, 128, KS * 128], bf, kind="ExternalInput")
    wk_d = nc.dram_tensor("wk", [128, KS * DKV], bf, kind="ExternalInput")
    wv_d = nc.dram_tensor("wv", [128, KS * DKV], bf, kind="ExternalInput")
    wo_d = nc.dram_tensor("wo", [4, 128, KS * 512], bf, kind="ExternalInput")
    bq_d = nc.dram_tensor("bq", [128, H], f32, kind="ExternalInput")
    bk_d = nc.dram_tensor("bk", [128, KVH], f32, kind="ExternalInput")
    bv_d = nc.dram_tensor("bv", [128, DKV], f32, kind="ExternalInput")
    cos_d = nc.dram_tensor("cosq", [128, SQ], bf, kind="ExternalInput")
    sin_d = nc.dram_tensor("sinq", [128, SQ], bf, kind="ExternalInput")
    ones_d = nc.dram_tensor("ones", [128, 128], bf, kind="ExternalInput")
    out_d = nc.dram_tensor("out", [SQ, D], f32, kind="ExternalOutput")

    # four quarter-AllGathers, one per kv head (0.25MB/rank): blocks 0-3 =
    # that head's KT [128,512], blocks 4-7 = its V for the 4 seq sub-chunks.
    # Small first AG => gathered K/V for heads 0-3 lands long before the
    # attention phase; the 4 gathers pipeline on the CC engine.
    sends = [nc.dram_tensor(f"send{i}", [8, 128, 128], bf) for i in range(KVH)]
    fulls = [nc.dram_tensor(f"full{i}", [G * 8, 128, 128], bf) for i in range(KVH)]
    RG = [[0, 1, 2, 3], [4, 5, 6, 7]]

    with tile.TileContext(nc) as tc, ExitStack() as ctx:
        const = ctx.enter_context(tc.tile_pool(name="const", bufs=1))
        big = ctx.enter_context(tc.tile_pool(name="big", bufs=1))
        wqp = ctx.enter_context(tc.tile_pool(name="wqp", bufs=5))
        wop = ctx.enter_context(tc.tile_pool(name="wop", bufs=3))
        fp = ctx.enter_context(tc.tile_pool(name="fp", bufs=2))
        rp = ctx.enter_context(tc.tile_pool(name="rp", bufs=3))
        ptp = ctx.enter_context(tc.tile_pool(name="ptp", bufs=4))
        sp = ctx.enter_context(tc.tile_pool(name="sp", bufs=3))
        tp2 = ctx.enter_context(tc.tile_pool(name="tp2", bufs=3))
        up = ctx.enter_context(tc.tile_pool(name="up", bufs=2))
        rootp = ctx.enter_context(tc.tile_pool(name="rootp", bufs=2))
        outp = ctx.enter_context(tc.tile_pool(name="outp", bufs=2))
        recs = ctx.enter_context(tc.tile_pool(name="recs", bufs=2))
        pp_proj = ctx.enter_context(tc.tile_pool(name="pp_proj", bufs=2, space="PSUM"))
        pp_st = ctx.enter_context(tc.tile_pool(name="pp_st", bufs=2, space="PSUM"))
        pp_av = ctx.enter_context(tc.tile_pool(name="pp_av", bufs=2, space="PSUM"))

        # ---------- consts + PE warm-up ----------
        # consts go out on the vector queue so sync/scalar queues are free
        # for the big input tiles below.
        cos_sb = const.tile([128, SQ], bf)
        sin_sb = const.tile([128, SQ], bf)
        bk_sb = const.tile([128, KVH], f32)
        bv_sb = const.tile([128, DKV], f32)
        bq_sb = const.tile([128, H], f32)
        ones_sb = const.tile([128, 128], bf)
        # ALL loads ride the sync queue: the scalar queue must stay clear so
        # rope Identity ops (critical path to the K/V sends) dispatch
        # immediately, and gpsimd must stay clear for the sends/AG triggers.
        nc.sync.dma_start(cos_sb[:], cos_d.ap())
        nc.sync.dma_start(sin_sb[:], sin_d.ap())
        nc.sync.dma_start(bk_sb[:], bk_d.ap())

        # dummy matmuls on zeroed SBUF while the first input DMAs land:
        # the PE clock gate (HAM) needs ~3.4us of sustained activity to
        # release full clock, so real matmuls start warm.
        scratch = const.tile([128, SQ], bf)
        nc.vector.memset(scratch[:], 0.0)
        warm_ps = pp_proj.tile([128, SQ], f32, tag="proj", name="warm_ps")
        for _ in range(14):
            nc.tensor.matmul(warm_ps[:], scratch[:, 0:128], scratch[:], start=True, stop=True)

        # ---------- staged input loads ----------
        # one trigger per tile: a queue's descriptors spread across all 16
        # DMA engines, so chunking only wastes sequencer descriptor-gen
        # time (~0.6us per trigger) — trigger count is the scarce resource.
        # Tiles stay moderately fine so deps release early.
        xt4 = [big.tile([128, 4 * SQ], bf, name=f"xt4_{i}") for i in range(4)]
        wk2 = [big.tile([128, 8 * DKV], bf, name=f"wk2_{i}") for i in range(2)]
        wv2 = [big.tile([128, 8 * DKV], bf, name=f"wv2_{i}") for i in range(2)]
        nc.sync.dma_start(wk2[0][:], wk_d.ap()[:, 0:8 * DKV])
        nc.sync.dma_start(xt4[0][:], xt_d.ap()[:, 0:4 * SQ])
        nc.sync.dma_start(xt4[1][:], xt_d.ap()[:, 4 * SQ:8 * SQ])
        nc.sync.dma_start(wk2[1][:], wk_d.ap()[:, 8 * DKV:16 * DKV])
        nc.sync.dma_start(xt4[2][:], xt_d.ap()[:, 8 * SQ:12 * SQ])
        nc.sync.dma_start(xt4[3][:], xt_d.ap()[:, 12 * SQ:16 * SQ])
        nc.sync.dma_start(bv_sb[:], bv_d.ap())
        nc.sync.dma_start(wv2[0][:], wv_d.ap()[:, 0:8 * DKV])
        nc.sync.dma_start(wv2[1][:], wv_d.ap()[:, 8 * DKV:16 * DKV])
        nc.sync.dma_start(bq_sb[:], bq_d.ap())
        nc.sync.dma_start(ones_sb[:], ones_d.ap())

        def xts(ks):
            return xt4[ks // 4][:, (ks % 4) * SQ:(ks % 4 + 1) * SQ]

        def rope(ps, bias_col, dst, swap_eng=None):
            # rotate-half form, all ops full-width and partition-aligned:
            # out = q*[cos;cos] + swap(q)*[-sin;sin] with swap via SBUF DMA.
            # bf16 throughout for the 2x DVE mode.
            if swap_eng is None:
                swap_eng = nc.sync
            qf = fp.tile([128, SQ], bf, tag="f")
            nc.scalar.activation(qf[:], ps[:], AF.Identity, bias=bias_col)
            qsw = fp.tile([128, SQ], bf, tag="fsw")
            swap_eng.dma_start(qsw[0:64, :], qf[64:128, :])
            swap_eng.dma_start(qsw[64:128, :], qf[0:64, :])
            ta = rp.tile([128, SQ], bf, tag="rt")
            nc.vector.tensor_mul(ta[:], qf[:], cos_sb[:])
            tb = rp.tile([128, SQ], bf, tag="rt")
            nc.vector.tensor_mul(tb[:], qsw[:], sin_sb[:])
            nc.vector.tensor_add(dst, ta[:], tb[:])

        # ---------- K/V projection for own chunk, RoPE(K), send ----------
        # order: K heads 0-1 -> V (all) -> AG1 fires early -> K heads 2-3 -> AG2
        # per-dt/per-st tiles so each send's dependency releases as soon as
        # its own projection finishes (deps are tile-granular); K-path rope
        # swaps ride the gpsimd queue, which is empty before the sends — on
        # sync/scalar they'd sit behind the bulk weight loads and delay the
        # AllGather doorbell (exec starts when the slowest rank is ready).
        kt_own4 = [big.tile([128, SQ], bf, name=f"kt_own{i}") for i in range(KVH)]
        v_own4 = [big.tile([128, DKV], bf, name=f"v_own{i}") for i in range(G)]

        def kproj(dt):
            ps = pp_proj.tile([128, SQ], f32, tag="proj", name=f"kps{dt}")
            for ks in range(KS):
                nc.tensor.matmul(
                    ps[:],
                    wk2[ks // 8][:, (ks % 8) * DKV + dt * 128:(ks % 8) * DKV + (dt + 1) * 128],
                    xts(ks),
                    start=(ks == 0), stop=(ks == KS - 1),
                )
            rope(ps, bk_sb[:, dt:dt + 1], kt_own4[dt][:], swap_eng=nc.gpsimd)

        # separate gathered-KV tiles per kv head (deps are tile-granular),
        # kt/kv-major layouts: column kt*128 holds key-tile kt.
        ktf4 = [big.tile([128, S], bf, name=f"ktf{i}") for i in range(KVH)]
        vf4 = [big.tile([128, S], bf, name=f"vf{i}") for i in range(KVH)]

        def kv_loads(kv):
            # coalesced: per source rank one trigger for K, one for V
            # (block-transposed dram APs) — 8 triggers per gather.
            full_d = fulls[kv]
            for g in range(G):
                src = full_d.ap()[g * 8: g * 8 + 4].transpose([1, 0, 2])
                nc.gpsimd.dma_start(ktf4[kv][:, g * SQ:(g + 1) * SQ], src)
                src = full_d.ap()[g * 8 + 4: g * 8 + 8].transpose([1, 0, 2])
                nc.gpsimd.dma_start(vf4[kv][:, g * SQ:(g + 1) * SQ], src)

        def kv_sends(kv):
            # 5 triggers: this head's KT [128,512] as 4 blocks, plus its V
            # slice from each of the 4 seq sub-chunks.
            nc.gpsimd.dma_start(sends[kv].ap()[0:4].transpose([1, 0, 2]), kt_own4[kv][:])
            for st in range(G):
                src = v_own4[st][:, kv * 128:(kv + 1) * 128]
                nc.gpsimd.dma_start(sends[kv].ap()[4 + st], src)

        for dt in (0, 1):
            kproj(dt)
        for st in range(G):
            ps = pp_proj.tile([128, DKV], f32, tag="proj")
            for ks in range(KS):
                nc.tensor.matmul(
                    ps[:],
                    xts(ks)[:, st * 128: st * 128 + 128],
                    wv2[ks // 8][:, (ks % 8) * DKV:(ks % 8 + 1) * DKV],
                    start=(ks == 0), stop=(ks == KS - 1),
                )
            nc.vector.tensor_add(v_own4[st][:], ps[:], bv_sb[:])
        # all AG triggers go out before any gather loads: the load
        # descriptors wait on AG completion and head-block the gpsimd DMA
        # queue, so anything queued behind them is delayed past that AG.
        def ag(kv):
            kv_sends(kv)
            nc.gpsimd.collective_compute(
                "AllGather", mybir.AluOpType.bypass,
                ins=[sends[kv].ap()], outs=[fulls[kv].ap()], replica_groups=RG,
            )
        ag(0)
        ag(1)

        # ---------- Q projection + RoPE (overlaps AllGather) ----------
        qt4 = [big.tile([128, 4 * SQ], bf, name=f"qt4_{i}") for i in range(4)]  # [p=hd, (h%4)*SQ + q]

        def qproj(ht):
            wq_t = wqp.tile([128, KS * 128], bf, tag="wq")
            nc.sync.dma_start(wq_t[:], wq_d.ap()[ht])
            ps = pp_proj.tile([128, SQ], f32, tag="proj")
            for ks in range(KS):
                nc.tensor.matmul(
                    ps[:],
                    wq_t[:, ks * 128:(ks + 1) * 128],
                    xts(ks),
                    start=(ks == 0), stop=(ks == KS - 1),
                )
            rope(ps, bq_sb[:, ht:ht + 1], qt4[ht // 4][:, (ht % 4) * SQ:(ht % 4 + 1) * SQ])

        # heads 0-3 first; kv heads 2/3 project + gather only after that, so
        # their AG mesh windows land over the (DMA-quiet) attention phase
        # instead of starving the wq stream — they aren't consumed until the
        # second half of attention.
        for ht in range(4):
            qproj(ht)
        for dt in (2, 3):
            kproj(dt)
        ag(2)
        ag(3)
        for kv in range(KVH):
            kv_loads(kv)
        for ht in range(4, H):
            qproj(ht)

        # ---------- attention per head ----------
        # Per k-tile pair: 2 score matmuls into one 2-bank PSUM tile, one
        # paired exp on ScalarE, AV matmuls one pair behind. Softmax
        # denominator from a DVE bf16 add-tree + one ones-matmul per head;
        # normalization (den/recip/mul) for head h is deferred into head
        # h+1 so the PE never waits on the DVE tree tail.
        a_sb = big.tile([128, H * SQ], bf)       # [p=hd, h*SQ + q]  (AV^T, normalized)
        pending = None

        def flush_pending():
            nonlocal pending
            if pending is None:
                return
            av_p, root_p, h_p = pending
            den = pp_proj.tile([128, SQ], f32, tag="proj", name=f"den{h_p}")
            nc.tensor.matmul(den[:], ones_sb[:], root_p[:], start=True, stop=True)
            recb = recs.tile([128, SQ], f32, tag="recb")
            nc.vector.reciprocal_approx_fast(recb[:], den[:])
            nc.vector.tensor_mul(a_sb[:, h_p * SQ:(h_p + 1) * SQ], av_p[:], recb[:])
            pending = None

        for h in range(H):
            kv = h // R
            ktf_t, vf_t = ktf4[kv], vf4[kv]
            av = pp_av.tile([128, SQ], f32, tag="av")
            pts = [None] * NPR
            svals = []
            tvals = []
            uvals = []
            root = None

            def av_pair(p):
                for kk in (0, 1):
                    kt = 2 * p + kk
                    nc.tensor.matmul(
                        av[:],
                        vf_t[:, kt * 128:(kt + 1) * 128],
                        pts[p][:, kk * SQ:(kk + 1) * SQ],
                        start=(kt == 0), stop=(kt == NKT - 1),
                    )

            for p in range(NPR):
                st_ps = pp_st.tile([128, 2 * SQ], f32, tag="st")
                for kk in (0, 1):
                    kt = 2 * p + kk
                    nc.tensor.matmul(
                        st_ps[:, kk * SQ:(kk + 1) * SQ],
                        ktf_t[:, kt * 128:(kt + 1) * 128],
                        qt4[h // 4][:, (h % 4) * SQ:(h % 4 + 1) * SQ],
                        start=True, stop=True,
                    )
                pt = ptp.tile([128, 2 * SQ], bf, tag="pt")
                nc.scalar.activation(pt[:], st_ps[:], AF.Exp, scale=SCALE)
                pts[p] = pt
                # denominator add-tree (all bf16 SBUF -> 2x DVE mode)
                s = sp.tile([128, SQ], bf, tag="s")
                nc.vector.tensor_add(s[:], pt[:, 0:SQ], pt[:, SQ:2 * SQ])
                svals.append(s)
                if p % 2 == 1:
                    t = tp2.tile([128, SQ], bf, tag="t")
                    nc.vector.tensor_add(t[:], svals[p - 1][:], svals[p][:])
                    tvals.append(t)
                if p % 4 == 3:
                    u = up.tile([128, SQ], bf, tag="u")
                    nc.vector.tensor_add(u[:], tvals[-2][:], tvals[-1][:])
                    uvals.append(u)
                if p >= 1:
                    av_pair(p - 1)
                if p == 1:
                    flush_pending()
            av_pair(NPR - 1)
            root = rootp.tile([128, SQ], bf, tag="root")
            nc.vector.tensor_add(root[:], uvals[0][:], uvals[1][:])
            pending = (av, root, h)
        flush_pending()

        # ---------- output projection ----------
        # wo prefetched as half-nt tiles on the sync queue during the
        # attention phase; output stores go out on the scalar queue.
        for nt in range(4):
            woh = []
            for i in range(2):
                wt = wop.tile([128, 8 * 512], bf, tag="wo")
                with tc.tile_wait_until(0.200):
                    nc.sync.dma_start(wt[:], wo_d.ap()[nt][:, 8 * i * 512:(8 * i + 8) * 512])
                woh.append(wt)
            for qt in range(4):
                ps = pp_proj.tile([128, 512], f32, tag="proj")
                for ct in range(KS):
                    nc.tensor.matmul(
                        ps[:],
                        a_sb[:, ct * SQ + qt * 128: ct * SQ + qt * 128 + 128],
                        woh[ct // 8][:, (ct % 8) * 512:(ct % 8 + 1) * 512],
                        start=(ct == 0), stop=(ct == KS - 1),
                    )
                ot = outp.tile([128, 512], f32, tag="ot")
                nc.scalar.activation(ot[:], ps[:], AF.Copy)
                nc.scalar.dma_start(out_d.ap()[qt * 128:(qt + 1) * 128, nt * 512:nt * 512 + 256], ot[:, 0:256])
                nc.scalar.dma_start(out_d.ap()[qt * 128:(qt + 1) * 128, nt * 512 + 256:(nt + 1) * 512], ot[:, 256:512])

    nc.compile()
    return nc


def get_nc():
    if "nc" not in _CACHE:
        _CACHE["nc"] = _build_nc()
    return _CACHE["nc"]


def make_in_maps(x, wq, bq, wk, bk, wv, bv, wo):
    bf16 = ml_dtypes.bfloat16
    perm = np.concatenate([np.arange(0, HD, 2), np.arange(1, HD, 2)])
    qcols = np.concatenate([h * HD + perm for h in range(H)])
    kcols = np.concatenate([h * HD + perm for h in range(KVH)])
    wq_p = wq[:, qcols]
    bq_p = np.ascontiguousarray(bq[qcols].reshape(H, HD).T).astype(np.float32)
    wk_p = wk[:, kcols]
    bk_p = np.ascontiguousarray(bk[kcols].reshape(KVH, HD).T).astype(np.float32)
    # pretile so every DMA is contiguous: wq [ht][p][ks][c], wk/wv [p][ks][c],
    # wo [nt][p][ct][c]
    wq_t = np.ascontiguousarray(
        wq_p.reshape(KS, 128, H, 128).transpose(2, 1, 0, 3).reshape(H, 128, KS * 128)
    ).astype(bf16)
    wk_t = np.ascontiguousarray(
        wk_p.reshape(KS, 128, DKV).transpose(1, 0, 2).reshape(128, KS * DKV)
    ).astype(bf16)
    wv_t = np.ascontiguousarray(
        wv.reshape(KS, 128, DKV).transpose(1, 0, 2).reshape(128, KS * DKV)
    ).astype(bf16)
    wo_t = np.ascontiguousarray(
        wo.reshape(KS, 128, 4, 512).transpose(2, 1, 0, 3).reshape(4, 128, KS * 512)
    ).astype(bf16)
    bv_rep = np.tile(bv.astype(np.float32), (128, 1))
    theta = (10000.0 ** (-np.arange(64, dtype=np.float64) / 64.0))
    ang = np.outer(np.arange(S, dtype=np.float64), theta)  # [S, 64]
    c = np.cos(ang).T.astype(np.float32)  # [64, S]
    s = np.sin(ang).T.astype(np.float32)
    cosT = np.concatenate([c, c], axis=0)      # [128, S]
    sinT = np.concatenate([-s, s], axis=0)     # [128, S]
    ones = np.ones((128, 128), dtype=bf16)

    in_maps = []
    for b in range(B):
        for g in range(G):
            sl = slice(g * SQ, (g + 1) * SQ)
            xt_c = np.ascontiguousarray(
                x[b, sl, :].T.reshape(KS, 128, SQ).transpose(1, 0, 2).reshape(128, KS * SQ)
            ).astype(bf16)
            in_maps.append({
                "xt": xt_c,
                "wq": wq_t, "wk": wk_t, "wv": wv_t, "wo": wo_t,
                "bq": bq_p, "bk": bk_p, "bv": bv_rep,
                "cosq": np.ascontiguousarray(cosT[:, sl]).astype(bf16),
                "sinq": np.ascontiguousarray(sinT[:, sl]).astype(bf16),
                "ones": ones,
            })
    return in_maps


def assemble(results):
    out = np.empty((B, S, D), np.float32)
    for b in range(B):
        for g in range(G):
            out[b, g * SQ:(g + 1) * SQ, :] = results[b * G + g]["out"]
    return out


def kernel(x, wq, bq, wk, bk, wv, bv, wo):
    from concourse.bass_utils import run_bass_kernel_spmd

    x, wq, bq, wk, bk, wv, bv, wo = (
        np.asarray(t, dtype=np.float32) for t in (x, wq, bq, wk, bk, wv, bv, wo)
    )
    nc = get_nc()
    in_maps = make_in_maps(x, wq, bq, wk, bk, wv, bv, wo)
    # run twice and return the second result: the first execution after a
    # NEFF load has occasionally produced stale collective output.
    run_bass_kernel_spmd(nc, in_maps, core_ids=list(range(NCORES)))
    res = run_bass_kernel_spmd(nc, in_maps, core_ids=list(range(NCORES)))
    return assemble(res.results)


# revision 53
# speedup vs baseline: 1.0009x; 1.0009x over previous
"""Multi-head GQA attention (B=2, S=2048, D=2048, H=16, KVH=4) on 8 TRN2
NeuronCores.

Sharding: core i = (b, g) with b = i // 4 (batch), g = i % 4 (sequence
chunk of 512 queries). Each core computes Q for its 512 queries over all
16 heads, K/V for its own 512 sequence positions, AllGathers K/V within
its 4-core batch group, then runs full attention + output projection for
its query chunk. Host concatenates the 8 [512, 2048] chunks.

Layout strategy (no on-chip transposes):
 - host passes x transposed per chunk (xT [D, 512]) so projections
   computed as w.T @ xT yield QT/KT with head-dim on partitions —
   exactly the operand layout attention needs.
 - wq/wk columns permuted per head (even dims first, odd second) so RoPE
   halves are contiguous partition ranges [0:64)/[64:128). Scores are
   permutation-invariant since q and k are permuted identically.
 - scores computed transposed (ST[k, q] = KT.T @ QT), exp'd on ScalarE
   straight out of PSUM two k-tiles at a time (scale=1/sqrt(HD) folded
   in, no max-subtraction: scores are O(10) so f32 exp is safe), giving
   probs in the [k, q] layout the AV matmul wants as its moving operand.
 - softmax denominator: DVE pairwise add-tree over the 16 prob tiles
   (bf16, 2x mode), then a single all-ones [k,128] stationary matmul on
   the tree root replicates the denominator across partitions, so
   normalization is reciprocal + elementwise multiply, no broadcast.
   (One matmul per head instead of 16 — the PE is the bottleneck.)
 - weights are host-pretiled so every DMA is a contiguous block.
 - a short stream of dummy matmuls on zeroed SBUF warms the PE clock
   gate (HAM) during the initial input DMAs, so real matmuls start at
   full clock.
"""

import numpy as np
import ml_dtypes

B, S, D = 2, 2048, 2048
H, KVH = 16, 4
HD = D // H            # 128
R = H // KVH           # 4 (GQA repeat)
NCORES = 8
G = 4                  # cores per batch group = seq chunks
SQ = S // G            # 512 queries/keys per core chunk
DKV = KVH * HD         # 512
KS = D // 128          # 16 contraction slices
NKT = S // 128         # 16 key tiles
NPR = NKT // 2         # 8 key-tile pairs
SCALE = 1.0 / float(np.sqrt(HD))

_CACHE = {}


def _build_nc():
    import concourse.tile as tile
    from concourse import bacc, mybir
    from contextlib import ExitStack

    f32 = mybir.dt.float32
    bf = mybir.dt.bfloat16
    AF = mybir.ActivationFunctionType

    nc = bacc.Bacc("TRN2", target_bir_lowering=False, debug=False, num_devices=NCORES)

    xt_d = nc.dram_tensor("xt", [128, KS * SQ], bf, kind="ExternalInput")
    wq_d = nc.dram_tensor("wq", [H, 128, KS * 128], bf, kind="ExternalInput")
    wk_d = nc.dram_tensor("wk", [128, KS * DKV], bf, kind="ExternalInput")
    wv_d = nc.dram_tensor("wv", [128, KS * DKV], bf, kind="ExternalInput")
    wo_d = nc.dram_tensor("wo", [4, 128, KS * 512], bf, kind="ExternalInput")
    bq_d = nc.dram_tensor("bq", [128, H], f32, kind="ExternalInput")
    bk_d = nc.dram_tensor("bk", [128, KVH], f32, kind="ExternalInput")
    bv_d = nc.dram_tensor("bv", [128, DKV], f32, kind="ExternalInput")
    cos_d = nc.dram_tensor("cosq", [128, SQ], bf, kind="ExternalInput")
    sin_d = nc.dram_tensor("sinq", [128, SQ], bf, kind="ExternalInput")
    ones_d = nc.dram_tensor("ones", [128, 128], bf, kind="ExternalInput")
    out_d = nc.dram_tensor("out", [SQ, D], f32, kind="ExternalOutput")

    # four quarter-AllGathers, one per kv head (0.25MB/rank): blocks 0-3 =
    # that head's KT [128,512], blocks 4-7 = its V for the 4 seq sub-chunks.
    # Small first AG => gathered K/V for heads 0-3 lands long before the
    # attention phase; the 4 gathers pipeline on the CC engine.
    sends = [nc.dram_tensor(f"send{i}", [8, 128, 128], bf) for i in range(KVH)]
    fulls = [nc.dram_tensor(f"full{i}", [G * 8, 128, 128], bf) for i in range(KVH)]
    RG = [[0, 1, 2, 3], [4, 5, 6, 7]]

    with tile.TileContext(nc) as tc, ExitStack() as ctx:
        const = ctx.enter_context(tc.tile_pool(name="const", bufs=1))
        big = ctx.enter_context(tc.tile_pool(name="big", bufs=1))
        wqp = ctx.enter_context(tc.tile_pool(name="wqp", bufs=5))
        wop = ctx.enter_context(tc.tile_pool(name="wop", bufs=3))
        fp = ctx.enter_context(tc.tile_pool(name="fp", bufs=2))
        rp = ctx.enter_context(tc.tile_pool(name="rp", bufs=3))
        ptp = ctx.enter_context(tc.tile_pool(name="ptp", bufs=4))
        sp = ctx.enter_context(tc.tile_pool(name="sp", bufs=3))
        tp2 = ctx.enter_context(tc.tile_pool(name="tp2", bufs=3))
        up = ctx.enter_context(tc.tile_pool(name="up", bufs=2))
        rootp = ctx.enter_context(tc.tile_pool(name="rootp", bufs=2))
        outp = ctx.enter_context(tc.tile_pool(name="outp", bufs=2))
        recs = ctx.enter_context(tc.tile_pool(name="recs", bufs=2))
        pp_proj = ctx.enter_context(tc.tile_pool(name="pp_proj", bufs=2, space="PSUM"))
        pp_st = ctx.enter_context(tc.tile_pool(name="pp_st", bufs=2, space="PSUM"))
        pp_av = ctx.enter_context(tc.tile_pool(name="pp_av", bufs=2, space="PSUM"))

        # ---------- consts + PE warm-up ----------
        # consts go out on the vector queue so sync/scalar queues are free
        # for the big input tiles below.
        cos_sb = const.tile([128, SQ], bf)
        sin_sb = const.tile([128, SQ], bf)
        bk_sb = const.tile([128, KVH], f32)
        bv_sb = const.tile([128, DKV], f32)
        bq_sb = const.tile([128, H], f32)
        ones_sb = const.tile([128, 128], bf)
        # ALL loads ride the sync queue: the scalar queue must stay clear so
        # rope Identity ops (critical path to the K/V sends) dispatch
        # immediately, and gpsimd must stay clear for the sends/AG triggers.
        nc.sync.dma_start(cos_sb[:], cos_d.ap())
        nc.sync.dma_start(sin_sb[:], sin_d.ap())
        nc.sync.dma_start(bk_sb[:], bk_d.ap())

        # dummy matmuls on zeroed SBUF while the first input DMAs land:
        # the PE clock gate (HAM) needs ~3.4us of sustained activity to
        # release full clock, so real matmuls start warm.
        scratch = const.tile([128, SQ], bf)
        nc.vector.memset(scratch[:], 0.0)
        warm_ps = pp_proj.tile([128, SQ], f32, tag="proj", name="warm_ps")
        for _ in range(14):
            nc.tensor.matmul(warm_ps[:], scratch[:, 0:128], scratch[:], start=True, stop=True)

        # ---------- staged input loads ----------
        # one trigger per tile: a queue's descriptors spread across all 16
        # DMA engines, so chunking only wastes sequencer descriptor-gen
        # time (~0.6us per trigger) — trigger count is the scarce resource.
        # Tiles stay moderately fine so deps release early.
        xt4 = [big.tile([128, 4 * SQ], bf, name=f"xt4_{i}") for i in range(4)]
        wk2 = [big.tile([128, 8 * DKV], bf, name=f"wk2_{i}") for i in range(2)]
        wv2 = [big.tile([128, 8 * DKV], bf, name=f"wv2_{i}") for i in range(2)]
        nc.sync.dma_start(wk2[0][:], wk_d.ap()[:, 0:8 * DKV])
        nc.sync.dma_start(xt4[0][:], xt_d.ap()[:, 0:4 * SQ])
        nc.sync.dma_start(xt4[1][:], xt_d.ap()[:, 4 * SQ:8 * SQ])
        nc.sync.dma_start(wk2[1][:], wk_d.ap()[:, 8 * DKV:16 * DKV])
        nc.sync.dma_start(xt4[2][:], xt_d.ap()[:, 8 * SQ:12 * SQ])
        nc.sync.dma_start(xt4[3][:], xt_d.ap()[:, 12 * SQ:16 * SQ])
        nc.sync.dma_start(bv_sb[:], bv_d.ap())
        nc.sync.dma_start(wv2[0][:], wv_d.ap()[:, 0:8 * DKV])
        nc.sync.dma_start(wv2[1][:], wv_d.ap()[:, 8 * DKV:16 * DKV])
        nc.sync.dma_start(bq_sb[:], bq_d.ap())
        nc.sync.dma_start(ones_sb[:], ones_d.ap())

        def xts(ks):
            return xt4[ks // 4][:, (ks % 4) * SQ:(ks % 4 + 1) * SQ]

        def rope(ps, bias_col, dst, swap_eng=None):
            # rotate-half form, all ops full-width and partition-aligned:
            # out = q*[cos;cos] + swap(q)*[-sin;sin] with swap via SBUF DMA.
            # bf16 throughout for the 2x DVE mode.
            if swap_eng is None:
                swap_eng = nc.sync
            qf = fp.tile([128, SQ], bf, tag="f")
            nc.scalar.activation(qf[:], ps[:], AF.Identity, bias=bias_col)
            qsw = fp.tile([128, SQ], bf, tag="fsw")
            swap_eng.dma_start(qsw[0:64, :], qf[64:128, :])
            swap_eng.dma_start(qsw[64:128, :], qf[0:64, :])
            ta = rp.tile([128, SQ], bf, tag="rt")
            nc.vector.tensor_mul(ta[:], qf[:], cos_sb[:])
            tb = rp.tile([128, SQ], bf, tag="rt")
            nc.vector.tensor_mul(tb[:], qsw[:], sin_sb[:])
            nc.vector.tensor_add(dst, ta[:], tb[:])

        # ---------- K/V projection for own chunk, RoPE(K), send ----------
        # order: K heads 0-1 -> V (all) -> AG1 fires early -> K heads 2-3 -> AG2
        # per-dt/per-st tiles so each send's dependency releases as soon as
        # its own projection finishes (deps are tile-granular); K-path rope
        # swaps ride the gpsimd queue, which is empty before the sends — on
        # sync/scalar they'd sit behind the bulk weight loads and delay the
        # AllGather doorbell (exec starts when the slowest rank is ready).
        kt_own4 = [big.tile([128, SQ], bf, name=f"kt_own{i}") for i in range(KVH)]
        v_own4 = [big.tile([128, DKV], bf, name=f"v_own{i}") for i in range(G)]

        def kproj(dt):
            ps = pp_proj.tile([128, SQ], f32, tag="proj", name=f"kps{dt}")
            for ks in range(KS):
                nc.tensor.matmul(
                    ps[:],
                    wk2[ks // 8][:, (ks % 8) * DKV + dt * 128:(ks % 8) * DKV + (dt + 1) * 128],
                    xts(ks),
                    start=(ks == 0), stop=(ks == KS - 1),
                )
            rope(ps, bk_sb[:, dt:dt + 1], kt_own4[dt][:], swap_eng=nc.gpsimd)

        # separate gathered-KV tiles per kv head (deps are tile-granular),
        # kt/kv-major layouts: column kt*128 holds key-tile kt.
        ktf4 = [big.tile([128, S], bf, name=f"ktf{i}") for i in range(KVH)]
        vf4 = [big.tile([128, S], bf, name=f"vf{i}") for i in range(KVH)]

        def kv_loads(kv):
            # coalesced: per source rank one trigger for K, one for V
            # (block-transposed dram APs) — 8 triggers per gather.
            full_d = fulls[kv]
            for g in range(G):
                src = full_d.ap()[g * 8: g * 8 + 4].transpose([1, 0, 2])
                nc.gpsimd.dma_start(ktf4[kv][:, g * SQ:(g + 1) * SQ], src)
                src = full_d.ap()[g * 8 + 4: g * 8 + 8].transpose([1, 0, 2])
                nc.gpsimd.dma_start(vf4[kv][:, g * SQ:(g + 1) * SQ], src)

        def kv_sends(kv):
            # 5 triggers: this head's KT [128,512] as 4 blocks, plus its V
            # slice from each of the 4 seq sub-chunks.
            nc.gpsimd.dma_start(sends[kv].ap()[0:4].transpose([1, 0, 2]), kt_own4[kv][:])
            for st in range(G):
                src = v_own4[st][:, kv * 128:(kv + 1) * 128]
                nc.gpsimd.dma_start(sends[kv].ap()[4 + st], src)

        for dt in (0, 1):
            kproj(dt)
        for st in range(G):
            ps = pp_proj.tile([128, DKV], f32, tag="proj")
            for ks in range(KS):
                nc.tensor.matmul(
                    ps[:],
                    xts(ks)[:, st * 128: st * 128 + 128],
                    wv2[ks // 8][:, (ks % 8) * DKV:(ks % 8 + 1) * DKV],
                    start=(ks == 0), stop=(ks == KS - 1),
                )
            nc.vector.tensor_add(v_own4[st][:], ps[:], bv_sb[:])
        # all AG triggers go out before any gather loads: the load
        # descriptors wait on AG completion and head-block the gpsimd DMA
        # queue, so anything queued behind them is delayed past that AG.
        def ag(kv):
            kv_sends(kv)
            nc.gpsimd.collective_compute(
                "AllGather", mybir.AluOpType.bypass,
                ins=[sends[kv].ap()], outs=[fulls[kv].ap()], replica_groups=RG,
            )
        ag(0)
        ag(1)

        # ---------- Q projection + RoPE (overlaps AllGather) ----------
        qt4 = [big.tile([128, 4 * SQ], bf, name=f"qt4_{i}") for i in range(4)]  # [p=hd, (h%4)*SQ + q]

        def qproj(ht):
            wq_t = wqp.tile([128, KS * 128], bf, tag="wq")
            nc.sync.dma_start(wq_t[:], wq_d.ap()[ht])
            ps = pp_proj.tile([128, SQ], f32, tag="proj")
            for ks in range(KS):
                nc.tensor.matmul(
                    ps[:],
                    wq_t[:, ks * 128:(ks + 1) * 128],
                    xts(ks),
                    start=(ks == 0), stop=(ks == KS - 1),
                )
            rope(ps, bq_sb[:, ht:ht + 1], qt4[ht // 4][:, (ht % 4) * SQ:(ht % 4 + 1) * SQ])

        # heads 0-3 first; kv heads 2/3 project + gather only after that, so
        # their AG mesh windows land over the (DMA-quiet) attention phase
        # instead of starving the wq stream — they aren't consumed until the
        # second half of attention.
        for ht in range(4):
            qproj(ht)
        for dt in (2, 3):
            kproj(dt)
        ag(2)
        ag(3)
        for kv in range(KVH):
            kv_loads(kv)
        for ht in range(4, H):
            qproj(ht)

        # ---------- attention per head ----------
        # Per k-tile pair: 2 score matmuls into one 2-bank PSUM tile, one
        # paired exp on ScalarE, AV matmuls one pair behind. Softmax
        # denominator from a DVE bf16 add-tree + one ones-matmul per head;
        # normalization (den/recip/mul) for head h is deferred into head
        # h+1 so the PE never waits on the DVE tree tail.
        a_sb = big.tile([128, H * SQ], bf)       # [p=hd, h*SQ + q]  (AV^T, normalized)
        pending = None

        def flush_pending():
            nonlocal pending
            if pending is None:
                return
            av_p, root_p, h_p = pending
            den = pp_proj.tile([128, SQ], f32, tag="proj", name=f"den{h_p}")
            nc.tensor.matmul(den[:], ones_sb[:], root_p[:], start=True, stop=True)
            recb = recs.tile([128, SQ], f32, tag="recb")
            nc.vector.reciprocal_approx_fast(recb[:], den[:])
            nc.vector.tensor_mul(a_sb[:, h_p * SQ:(h_p + 1) * SQ], av_p[:], recb[:])
            pending = None

        for h in range(H):
            kv = h // R
            ktf_t, vf_t = ktf4[kv], vf4[kv]
            av = pp_av.tile([128, SQ], f32, tag="av")
            pts = [None] * NPR
            svals = []
            tvals = []
            uvals = []
            root = None

            def av_pair(p):
                for kk in (0, 1):
                    kt = 2 * p + kk
                    nc.tensor.matmul(
                        av[:],
                        vf_t[:, kt * 128:(kt + 1) * 128],
                        pts[p][:, kk * SQ:(kk + 1) * SQ],
                        start=(kt == 0), stop=(kt == NKT - 1),
                    )

            for p in range(NPR):
                st_ps = pp_st.tile([128, 2 * SQ], f32, tag="st")
                for kk in (0, 1):
                    kt = 2 * p + kk
                    nc.tensor.matmul(
                        st_ps[:, kk * SQ:(kk + 1) * SQ],
                        ktf_t[:, kt * 128:(kt + 1) * 128],
                        qt4[h // 4][:, (h % 4) * SQ:(h % 4 + 1) * SQ],
                        start=True, stop=True,
                    )
                pt = ptp.tile([128, 2 * SQ], bf, tag="pt")
                nc.scalar.activation(pt[:], st_ps[:], AF.Exp, scale=SCALE)
                pts[p] = pt
                # denominator add-tree (all bf16 SBUF -> 2x DVE mode)
                s = sp.tile([128, SQ], bf, tag="s")
                nc.vector.tensor_add(s[:], pt[:, 0:SQ], pt[:, SQ:2 * SQ])
                svals.append(s)
                if p % 2 == 1:
                    t = tp2.tile([128, SQ], bf, tag="t")
                    nc.vector.tensor_add(t[:], svals[p - 1][:], svals[p][:])
                    tvals.append(t)
                if p % 4 == 3:
                    u = up.tile([128, SQ], bf, tag="u")
                    nc.vector.tensor_add(u[:], tvals[-2][:], tvals[-1][:])
                    uvals.append(u)
                if p >= 1:
                    av_pair(p - 1)
                if p == 1:
                    flush_pending()
            av_pair(NPR - 1)
            root = rootp.tile([128, SQ], bf, tag="root")
            nc.vector.tensor_add(root[:], uvals[0][:], uvals[1][:])
            pending = (av, root, h)
        flush_pending()

        # ---------- output projection ----------
        # wo prefetched as half-nt tiles on the sync queue during the
        # attention phase; output stores go out on the scalar queue.
        for nt in range(4):
            woh = []
            for i in range(2):
                wt = wop.tile([128, 8 * 512], bf, tag="wo")
                with tc.tile_wait_until(0.150):
                    nc.sync.dma_start(wt[:], wo_d.ap()[nt][:, 8 * i * 512:(8 * i + 8) * 512])
                woh.append(wt)
            for qt in range(4):
                ps = pp_proj.tile([128, 512], f32, tag="proj")
                for ct in range(KS):
                    nc.tensor.matmul(
                        ps[:],
                        a_sb[:, ct * SQ + qt * 128: ct * SQ + qt * 128 + 128],
                        woh[ct // 8][:, (ct % 8) * 512:(ct % 8 + 1) * 512],
                        start=(ct == 0), stop=(ct == KS - 1),
                    )
                ot = outp.tile([128, 512], f32, tag="ot")
                nc.scalar.activation(ot[:], ps[:], AF.Copy)
                if nt == 3:
                    # final group: halve the store tail with both queues
                    nc.scalar.dma_start(out_d.ap()[qt * 128:(qt + 1) * 128, nt * 512:nt * 512 + 256], ot[:, 0:256])
                    nc.sync.dma_start(out_d.ap()[qt * 128:(qt + 1) * 128, nt * 512 + 256:(nt + 1) * 512], ot[:, 256:512])
                else:
                    nc.scalar.dma_start(out_d.ap()[qt * 128:(qt + 1) * 128, nt * 512:nt * 512 + 256], ot[:, 0:256])
                    nc.scalar.dma_start(out_d.ap()[qt * 128:(qt + 1) * 128, nt * 512 + 256:(nt + 1) * 512], ot[:, 256:512])

    nc.compile()
    return nc


def get_nc():
    if "nc" not in _CACHE:
        _CACHE["nc"] = _build_nc()
    return _CACHE["nc"]


def make_in_maps(x, wq, bq, wk, bk, wv, bv, wo):
    bf16 = ml_dtypes.bfloat16
    perm = np.concatenate([np.arange(0, HD, 2), np.arange(1, HD, 2)])
    qcols = np.concatenate([h * HD + perm for h in range(H)])
    kcols = np.concatenate([h * HD + perm for h in range(KVH)])
    wq_p = wq[:, qcols]
    bq_p = np.ascontiguousarray(bq[qcols].reshape(H, HD).T).astype(np.float32)
    wk_p = wk[:, kcols]
    bk_p = np.ascontiguousarray(bk[kcols].reshape(KVH, HD).T).astype(np.float32)
    # pretile so every DMA is contiguous: wq [ht][p][ks][c], wk/wv [p][ks][c],
    # wo [nt][p][ct][c]
    wq_t = np.ascontiguousarray(
        wq_p.reshape(KS, 128, H, 128).transpose(2, 1, 0, 3).reshape(H, 128, KS * 128)
    ).astype(bf16)
    wk_t = np.ascontiguousarray(
        wk_p.reshape(KS, 128, DKV).transpose(1, 0, 2).reshape(128, KS * DKV)
    ).astype(bf16)
    wv_t = np.ascontiguousarray(
        wv.reshape(KS, 128, DKV).transpose(1, 0, 2).reshape(128, KS * DKV)
    ).astype(bf16)
    wo_t = np.ascontiguousarray(
        wo.reshape(KS, 128, 4, 512).transpose(2, 1, 0, 3).reshape(4, 128, KS * 512)
    ).astype(bf16)
    bv_rep = np.tile(bv.astype(np.float32), (128, 1))
    theta = (10000.0 ** (-np.arange(64, dtype=np.float64) / 64.0))
    ang = np.outer(np.arange(S, dtype=np.float64), theta)  # [S, 64]
    c = np.cos(ang).T.astype(np.float32)  # [64, S]
    s = np.sin(ang).T.astype(np.float32)
    cosT = np.concatenate([c, c], axis=0)      # [128, S]
    sinT = np.concatenate([-s, s], axis=0)     # [128, S]
    ones = np.ones((128, 128), dtype=bf16)

    in_maps = []
    for b in range(B):
        for g in range(G):
            sl = slice(g * SQ, (g + 1) * SQ)
            xt_c = np.ascontiguousarray(
                x[b, sl, :].T.reshape(KS, 128, SQ).transpose(1, 0, 2).reshape(128, KS * SQ)
            ).astype(bf16)
            in_maps.append({
                "xt": xt_c,
                "wq": wq_t, "wk": wk_t, "wv": wv_t, "wo": wo_t,
                "bq": bq_p, "bk": bk_p, "bv": bv_rep,
                "cosq": np.ascontiguousarray(cosT[:, sl]).astype(bf16),
                "sinq": np.ascontiguousarray(sinT[:, sl]).astype(bf16),
                "ones": ones,
            })
    return in_maps


def assemble(results):
    out = np.empty((B, S, D), np.float32)
    for b in range(B):
        for g in range(G):
            out[b, g * SQ:(g + 1) * SQ, :] = results[b * G + g]["out"]
    return out


def kernel(x, wq, bq, wk, bk, wv, bv, wo):
    from concourse.bass_utils import run_bass_kernel_spmd

    x, wq, bq, wk, bk, wv, bv, wo = (
        np.asarray(t, dtype=np.float32) for t in (x, wq, bq, wk, bk, wv, bv, wo)
    )
    nc = get_nc()
    in_maps = make_in_maps(x, wq, bq, wk, bk, wv, bv, wo)
    # run twice and return the second result: the first execution after a
    # NEFF load has occasionally produced stale collective output.
    run_bass_kernel_spmd(nc, in_maps, core_ids=list(range(NCORES)))
    res = run_bass_kernel_spmd(nc, in_maps, core_ids=list(range(NCORES)))
    return assemble(res.results)
